# revision 23
# baseline (speedup 1.0000x reference)
"""AdaptiveSplineLayer on 8 Trainium2 NeuronCores (Bass/Tile).

Reference computation (per element, feature i, row m):
    sort grid[i], mc = (coeffs*sigmoid(alive)) sorted by grid order
    xn = clip((clip(x, gmin, gmax) - gmin) / range * 11, ...)
    spline = lerp of mc at floor(xn)
    out = spline @ proj_w.T + proj_b + x @ res_w.T

Kernel strategy (data-parallel over M; spline params + weights replicated):
  * Host ships TWO coordinate encodings of w = gscale*x + gbias:
      - w16 = fp16(clip(w, 0, 11)), feature-major: the spline coordinate.
        f(clip(w)) == f(w) exactly (the kink decomposition is flat outside),
        so kink 11 vanishes (relu(w16-11) == 0) and kink 0 degenerates to the
        pure linear term D_0*w16.
      - w8h/w8l = fp8 split of the UNCLAMPED centered (w-5.5)/8: the res-path
        operand (two fp8 digits; weights scaled by 8 keep fp8 normal-range).
  * Spline f(w) = mc0 + sum_j D_j relu(w-j), kinks split across engines:
      - kink 0: one 4x-mode tensor_scalar D_0*w16 (fp16) seeding the chain
      - kinks 1..8: 4 custom-DVE pair passes (fp32 chain, 2 kinks/pass),
        final pass writes the fp8 spl stream
      - kinks 9,10 (ACT3): ACT relu passes, scale=|D_j|*SPL (always the
        bounded minority-side orientation: D*relu(w-j) = sign(D)*
        relu(|D|*(w-j))), written directly as fp8 PE operand streams with
        sign(D) folded into their private weight copies
  * TensorE runs weights-stationary fp8 DoubleRow for all streams, producing
    out^T[o, m] in PSUM; groups ordered by dependency depth (res-path groups
    depend only on DMAs, so PE starts before the DVE chain ends).
    Evacuation (fp32->fp16 cast) runs on gpsimd/Pool (or ACT/DVE, flag) with
    NO bias: the folded bias is added on the host after the gather, where the
    fp16 output is upcast/transposed anyway.
"""

import os
import sys

import numpy as np

for _p in ("/opt/trn_rl_repo",):
    if _p not in sys.path and os.path.isdir(_p):
        sys.path.insert(0, _p)

import ml_dtypes

BF16 = ml_dtypes.bfloat16
FP8 = ml_dtypes.float8_e4m3fn
F16 = np.float16

M, IN, OUT, K = 16384, 1024, 1024, 12
N_CORES = 8
MC = M // N_CORES  # 2048 rows per core
FT = IN // 128  # 8 feature tiles
MT = MC // 128  # 16 m tiles per core
NKINK = K
SPL_SCALE = 4.0  # spline fp8 upscale (proj weights descaled to match)
W_CENTER = 5.5  # centering for the res-path fp8 split
RS = 8.0  # res-path weight upscale (keeps fp8 weights normal-range)

# On real TRN2, fp8-DoubleRow is only ~1.44x over fp16 (Ldweights +72%), so
# per-kink fp8 PE streams and the split-fp8 res path are net losses; the res
# path stays fp16 and all 10 surviving kinks (1..10) run as 5 DVE pair passes.
ACT3 = ()  # kinks on ACT with own fp8 PE streams — disabled (Ldweights cost)
DVE_QS = (1, 3, 5, 7, 9)  # custom-DVE pair starts (kinks q, q+1)

# default engine-routing config; override via KCFG env (json) for experiments
import json as _json

CFG = {
    "evac": "act",
    "seed_eng": "dve",
    "kink9": "act",
    "out_dma": "gpsimd",
    "in_dma2": "act",  # issue queue for the w16u DMAs: "act" | "sync"
}
CFG.update(_json.loads(os.environ.get("KCFG", "{}")))

# --------------------------------------------------------------------------
# Custom DVE ops: two relu-kinks per pass.
#   SPLINE_PAIR0   : out = s0*relu(in0-imm2) + s1*relu(in0-(imm2+1))
#   SPLINE_PAIR_ACC: out = in1 + s0*relu(in0-imm2) + s1*relu(in0-(imm2+1))
# --------------------------------------------------------------------------


def _register_spline_ops():
    from concourse.dve_ops import (
        CUSTOM_DVE_SPECS,
        OPS,
        _SUB_OPCODE_FOR_NAME,
        DveOp,
    )
    from concourse.dve_spec import (
        C0,
        C1,
        C2,
        One,
        Spec,
        Src0,
        Src1,
        _has_src1,
        lower,
        relu,
    )
    from concourse.dve_uop import DveOpSpec

    def _dve_relu(x):
        return np.maximum(
            np.nan_to_num(x, nan=0.0, posinf=np.inf, neginf=-np.inf), 0
        )

    def _ref_pair_acc(in0, in1, s0, s1, imm2):
        return (
            in1
            + s0 * _dve_relu(in0.astype(np.float32) - imm2)
            + s1 * _dve_relu(in0.astype(np.float32) - (imm2 + 1.0))
        ).astype(np.float32)

    def _ref_pair0(in0, in1, s0, s1, imm2):
        return (
            s0 * _dve_relu(in0.astype(np.float32) - imm2)
            + s1 * _dve_relu(in0.astype(np.float32) - (imm2 + 1.0))
        ).astype(np.float32)

    def _reg(name, spec):
        if name in _SUB_OPCODE_FOR_NAME:
            return next(o for o in OPS if o.name == name)
        row = max(_SUB_OPCODE_FOR_NAME.values()) + 1
        assert row < 0x20
        op = DveOp.__new__(DveOp)
        object.__setattr__(op, "name", name)
        object.__setattr__(op, "spec", spec)
        object.__setattr__(op, "subdim", False)
        object.__setattr__(op, "perf_en", {})
        s = DveOpSpec(
            name=name,
            opcode=row,
            uops=lower(spec, ver="v3"),
            rd1_en=_has_src1(spec),
        )
        object.__setattr__(op, "uops_sha", {"v3": s.sha("v3")})
        OPS.append(op)
        _SUB_OPCODE_FOR_NAME[name] = row
        CUSTOM_DVE_SPECS[name] = spec
        return op

    body_acc = (Src1 + C0 * relu(Src0 - C2)) + C1 * relu(Src0 - (C2 + One))
    body0 = C0 * relu(Src0 - C2) + C1 * relu(Src0 - (C2 + One))
    pair_acc = _reg("SPLINE_PAIR_ACC", Spec(body=body_acc, reference=_ref_pair_acc))
    pair0 = _reg("SPLINE_PAIR0", Spec(body=body0, reference=_ref_pair0))
    return pair0, pair_acc


# --------------------------------------------------------------------------
# Device graph
# --------------------------------------------------------------------------

_GRAPH_CACHE = {}


def _build_graph(
    m_split=(4, 4, 4, 4),
    repeat=1,
    acc_bufs=8,
    chunk_cols=512,
    stream_bufs=4,
    fori=False,
    skip=(),
    res_split=False,
    evac=None,  # "pool" | "act" | "dve"
    seed_eng=None,  # engine for the kink-0 linear seed: "dve" | "pool"
    kink9=None,  # "act" (fp8 PE stream) | "pool" (TS relu + STT merge on Pool)
    out_dma=None,  # "sync" (SP hwdge) | "gpsimd"
):
    evac = evac if evac is not None else CFG["evac"]
    seed_eng = seed_eng if seed_eng is not None else CFG["seed_eng"]
    kink9 = kink9 if kink9 is not None else CFG["kink9"]
    out_dma = out_dma if out_dma is not None else CFG["out_dma"]
    in2 = CFG["in_dma2"]
    if isinstance(m_split, (tuple, list)):
        m_split = tuple(m_split)
    key = (m_split, repeat, acc_bufs, chunk_cols, stream_bufs, fori,
           tuple(skip), res_split, evac, seed_eng, kink9, out_dma, in2)
    if key in _GRAPH_CACHE:
        return _GRAPH_CACHE[key]

    import concourse.bacc as bacc
    import concourse.mybir as mybir
    import concourse.tile as tile

    pair0, pair_acc = _register_spline_ops()

    dt = mybir.dt
    DR = mybir.MatmulPerfMode.DoubleRow
    Alu = mybir.AluOpType
    RELU = mybir.ActivationFunctionType.Relu
    act3 = ACT3 if kink9 == "act" else tuple(j for j in ACT3 if j != 9)
    N3 = len(ACT3)  # param tensors always sized for the full ACT3 set
    n3s = len(act3)  # streams actually fed to the PE
    nc = bacc.Bacc("TRN2", target_bir_lowering=False, debug=False, num_devices=1)

    w16 = nc.dram_tensor("w16", [IN, MC], dt.float16, kind="ExternalInput")
    wtb = nc.dram_tensor("wtb", [128, FT, OUT], dt.float8e4, kind="ExternalInput")
    if not res_split:
        w16u = nc.dram_tensor("w16u", [IN, MC], dt.float16, kind="ExternalInput")
    if res_split:
        w8h = nc.dram_tensor("w8h", [IN, MC], dt.float8e4, kind="ExternalInput")
        w8l = nc.dram_tensor("w8l", [IN, MC], dt.float8e4, kind="ExternalInput")
        rwh = nc.dram_tensor("rwh", [128, FT, OUT], dt.float8e4, kind="ExternalInput")
        rwl = nc.dram_tensor("rwl", [128, FT, OUT], dt.float8e4, kind="ExternalInput")
    else:
        wt8 = nc.dram_tensor("wt8", [128, FT, OUT], dt.float16, kind="ExternalInput")
    # per-kink PE-stream weights (sign folded in)
    if N3:
        wpe = nc.dram_tensor("wpe", [128, N3, FT, OUT], dt.float8e4, kind="ExternalInput")
        acts = nc.dram_tensor("acts", [128, FT, N3], dt.float32, kind="ExternalInput")
        actb = nc.dram_tensor("actb", [128, FT, N3], dt.float32, kind="ExternalInput")
    lin0 = nc.dram_tensor("lin0", [128, FT], dt.float32, kind="ExternalInput")
    dcoef = nc.dram_tensor("dcoef", [128, FT, NKINK], dt.float32, kind="ExternalInput")
    out = nc.dram_tensor("out", [OUT, MC], dt.float16, kind="ExternalOutput")

    assert len(set(m_split)) == 1, "equal blocks keep the stream pool uniform"
    MS = m_split[0] * 128

    with tile.TileContext(nc) as tc:
        with (
            tc.tile_pool(name="const", bufs=1) as const_pool,
            tc.tile_pool(name="wtp", bufs=1) as wt_pool,
            tc.tile_pool(name="stream", bufs=stream_bufs) as stream_pool,
            tc.tile_pool(name="up", bufs=4) as u_pool,
            tc.tile_pool(name="accp", bufs=acc_bufs) as acc_pool,
            tc.tile_pool(name="outp", bufs=4) as out_pool,
            tc.tile_pool(
                name="psum", bufs=(8 * 512) // chunk_cols, space="PSUM"
            ) as psum_pool,
        ):
            # ---- small constants first (first DVE op needs dcoef) ----
            dc_t = const_pool.tile([128, FT, NKINK], dt.float32)
            nc.sync.dma_start(dc_t[:], dcoef[:])
            if N3:
                acts_t = const_pool.tile([128, FT, N3], dt.float32)
                nc.sync.dma_start(acts_t[:], acts[:])
                actb_t = const_pool.tile([128, FT, N3], dt.float32)
                nc.sync.dma_start(actb_t[:], actb[:])
            lin0_t = const_pool.tile([128, FT], dt.float32)
            nc.sync.dma_start(lin0_t[:], lin0[:])

            wtb_t = wt_pool.tile([128, FT, OUT], dt.float8e4, tag="wtb")
            if N3:
                wpe_t = wt_pool.tile([128, N3, FT, OUT], dt.float8e4, tag="wpe")
            if res_split:
                rwh_t = wt_pool.tile([128, FT, OUT], dt.float8e4, tag="rwh")
                rwl_t = wt_pool.tile([128, FT, OUT], dt.float8e4, tag="rwl")
            else:
                wt8_t = wt_pool.tile([128, FT, OUT], dt.float16, tag="wt8")

            from contextlib import nullcontext

            def load_weights():
                nc.sync.dma_start(wtb_t[:], wtb[:])
                if N3:
                    nc.sync.dma_start(wpe_t[:], wpe[:])
                if res_split:
                    nc.sync.dma_start(rwh_t[:], rwh[:])
                    nc.sync.dma_start(rwl_t[:], rwl[:])
                else:
                    nc.sync.dma_start(wt8_t[:], wt8[:])

            if fori:
                # bench mode: weights are loop-invariant; load them up front
                load_weights()
            rep_ctx = (
                tc.For_i(0, repeat, 1, staggered_reset=True)
                if fori
                else nullcontext()
            )
            blocks = list(m_split)
            assert sum(blocks) == MT
            starts = [sum(blocks[:i]) for i in range(len(blocks))]
            with rep_ctx:
              for _rep in range(1 if fori else repeat):
                for h, (b0, bn) in enumerate(zip(starts, blocks)):
                    sl = slice(b0 * 128, (b0 + bn) * 128)
                    # per-block rotating stream tiles (double-buffered so the
                    # next block/iteration DMAs overlap this block matmuls)
                    w16_t = stream_pool.tile([128, FT, MS], dt.float16, tag="w16")
                    spl_t = stream_pool.tile([128, FT, MS], dt.float8e4, tag="spl")
                    if not res_split:
                        w16u_t = stream_pool.tile([128, FT, MS], dt.float16, tag="w16u")
                    if res_split:
                        w8h_t = stream_pool.tile([128, FT, MS], dt.float8e4, tag="w8h")
                        w8l_t = stream_pool.tile([128, FT, MS], dt.float8e4, tag="w8l")
                    if n3s:
                        u3_t = stream_pool.tile([128, n3s, FT, MS], dt.float8e4, tag="u3")
                    if "spline" in skip:
                        nc.gpsimd.memset(spl_t[:], 0.0)
                        if n3s:
                            nc.gpsimd.memset(u3_t[:], 0.0)
                    tpairs = (
                        [(t, t + 1) for t in range(0, FT, 2)]
                        if "spline" not in skip
                        else []
                    )
                    if "spline" in skip:
                        for t in range(FT):
                            nc.sync.dma_start(
                                w16_t[:, t, :], w16[128 * t : 128 * (t + 1), sl]
                            )
                            if res_split:
                                nc.sync.dma_start(
                                    w8h_t[:, t, :], w8h[128 * t : 128 * (t + 1), sl]
                                )
                                nc.sync.dma_start(
                                    w8l_t[:, t, :], w8l[128 * t : 128 * (t + 1), sl]
                                )
                            else:
                                in2_eng = nc.scalar if in2 == "act" else nc.sync
                                in2_eng.dma_start(
                                    w16u_t[:, t, :], w16u[128 * t : 128 * (t + 1), sl]
                                )
                    for tp in tpairs:
                        # input DMAs for this tile pair
                        for t in tp:
                            nc.sync.dma_start(
                                w16_t[:, t, :], w16[128 * t : 128 * (t + 1), sl]
                            )
                            if res_split:
                                nc.sync.dma_start(
                                    w8h_t[:, t, :], w8h[128 * t : 128 * (t + 1), sl]
                                )
                                nc.sync.dma_start(
                                    w8l_t[:, t, :], w8l[128 * t : 128 * (t + 1), sl]
                                )
                            else:
                                in2_eng = nc.scalar if in2 == "act" else nc.sync
                                in2_eng.dma_start(
                                    w16u_t[:, t, :], w16u[128 * t : 128 * (t + 1), sl]
                                )
                        # ACT3 kinks: u = relu(|Dd|*w16 - j*|Dd|) -> fp8 stream
                        for t in tp:
                            for a, j in enumerate(act3):
                                ai = ACT3.index(j)
                                nc.scalar.activation(
                                    u3_t[:, a, t, :], w16_t[:, t, :], RELU,
                                    bias=actb_t[:, t, ai : ai + 1],
                                    scale=acts_t[:, t, ai : ai + 1],
                                )
                        # kink-0 linear seed: Dd_0 * w16 (4x-mode tensor_scalar)
                        seeds = {}
                        for t in tp:
                            seng = nc.gpsimd if seed_eng == "pool" else nc.vector
                            sd = u_pool.tile([128, MS], dt.float16, tag="seed")
                            seng.tensor_scalar_mul(
                                sd[:], w16_t[:, t, :], lin0_t[:, t : t + 1]
                            )
                            seeds[t] = sd
                        if kink9 == "pool":
                            # kink 9 on Pool: r9 = relu(w16-9); seed += Dd9*r9
                            for t in tp:
                                r9 = u_pool.tile([128, MS], dt.float16, tag="r9")
                                nc.gpsimd.tensor_scalar(
                                    r9[:], w16_t[:, t, :], 9.0, 0.0,
                                    Alu.subtract, Alu.max,
                                )
                                sd2 = u_pool.tile([128, MS], dt.float16, tag="sd2")
                                nc.gpsimd.scalar_tensor_tensor(
                                    sd2[:], r9[:], dc_t[:, t, 9:10], seeds[t][:],
                                    Alu.mult, Alu.add,
                                )
                                seeds[t] = sd2
                        # custom-DVE pair chain (fp32 acc), seeded
                        accs = {}
                        for qi, q in enumerate(DVE_QS):
                            last = qi == len(DVE_QS) - 1
                            for t in tp:
                                dcs0 = dc_t[:, t, q : q + 1]
                                dcs1 = dc_t[:, t, q + 1 : q + 2]
                                if last:
                                    o = spl_t[:, t, :]
                                else:
                                    acc_nt = acc_pool.tile(
                                        [128, MS], dt.float32, tag="acc"
                                    )
                                    o = acc_nt[:]
                                in1 = seeds[t][:] if qi == 0 else accs[t]
                                nc.vector._custom_dve(
                                    pair_acc, out=o, in0=w16_t[:, t, :],
                                    in1=in1, s0=dcs0, s1=dcs1,
                                    imm2=float(q),
                                )
                                if not last:
                                    accs[t] = o
                    if h == 0 and _rep == 0 and not fori:
                        load_weights()
                    # matmuls, weights-stationary fp8 DoubleRow; out^T[o, m]
                    assert bn % 4 == 0, "blocks must align to 512-row chunks"
                    CW = chunk_cols
                    assert (bn * 128) % CW == 0
                    mchunks = [
                        slice(CW * c, CW * (c + 1))
                        for c in range(bn * 128 // CW)
                    ] if "mm" not in skip else []
                    for ob in range(FT):  # 8 output-column blocks of 128
                        opart = slice(128 * ob, 128 * (ob + 1))
                        if "mm" in skip:
                            break
                        # weight-stationary groups: (lhsT slice, rhs_fn).
                        # Order by dependency depth: res-path groups depend
                        # only on DMAs, u3 groups on ACT, spl groups on the
                        # DVE chain -- so PE can start before the chain ends.
                        groups = []
                        for q in range(FT // 2):
                            tsl = slice(2 * q, 2 * q + 2)

                            def _r(tens, tsl=tsl):
                                return lambda msl: tens[:, tsl, msl]

                            if res_split:
                                groups.append((rwh_t[:, tsl, opart], _r(w8h_t)))
                                groups.append((rwl_t[:, tsl, opart], _r(w8h_t)))
                                groups.append((rwh_t[:, tsl, opart], _r(w8l_t)))
                        for q in range(FT // 2):
                            tsl = slice(2 * q, 2 * q + 2)
                            for a, j in enumerate(act3):
                                ai = ACT3.index(j)
                                groups.append((
                                    wpe_t[:, ai, tsl, opart],
                                    lambda msl, a=a, tsl=tsl: u3_t[:, a, tsl, msl],
                                ))
                        for q in range(FT // 2):
                            tsl = slice(2 * q, 2 * q + 2)

                            def _r2(tens, tsl=tsl):
                                return lambda msl: tens[:, tsl, msl]

                            groups.append((wtb_t[:, tsl, opart], _r2(spl_t)))
                        if not res_split:
                            fp16_groups = [
                                (
                                    wt8_t[:, t, opart],
                                    lambda msl, t=t: w16u_t[:, t, msl],
                                )
                                for t in range(FT)
                            ]
                        # group-outer / chunk-inner: one Ldweights per
                        # (ob, group), amortized over all m-chunks and
                        # overlappable with the previous group's matmuls
                        pss = {}
                        for ci in range(len(mchunks)):
                            ps_t = psum_pool.tile([128, CW], dt.float32)
                            pss[ci] = ps_t
                        ng = len(groups) + (0 if res_split else FT)
                        gi = 0
                        if not res_split:
                            for lhsT, rhs_fn in fp16_groups:
                                for ci, msl in enumerate(mchunks):
                                    nc.tensor.matmul(
                                        pss[ci][:], lhsT, rhs_fn(msl),
                                        start=(gi == 0), stop=False,
                                    )
                                gi += 1
                        for lhsT, rhs_fn in groups:
                            for ci, msl in enumerate(mchunks):
                                nc.tensor.matmul(
                                    pss[ci][:], lhsT, rhs_fn(msl),
                                    start=(gi == 0), stop=(gi == ng - 1),
                                    perf_mode=DR,
                                )
                            gi += 1
                        # evacuate: cast fp32->fp16 (bias is added on host)
                        ot = out_pool.tile([128, bn * 128], dt.float16, tag="evac")
                        for ci, msl in enumerate(mchunks):
                            osl = slice(CW * ci, CW * (ci + 1))
                            if evac == "pool":
                                nc.gpsimd.tensor_scalar_add(
                                    ot[:, osl], pss[ci][:], 0.0
                                )
                            elif evac == "dve":
                                nc.vector.tensor_scalar_add(
                                    ot[:, osl], pss[ci][:], 0.0
                                )
                            else:
                                nc.scalar.activation(
                                    ot[:, osl], pss[ci][:],
                                    mybir.ActivationFunctionType.Identity,
                                )
                        dma_eng = nc.sync if out_dma == "sync" else nc.gpsimd
                        dma_eng.dma_start(
                            out[opart, b0 * 128 : (b0 + bn) * 128], ot[:]
                        )

            if "mm" in skip:
                with tc.tile_pool(name="sinkp", bufs=2) as sink_pool:
                    for ob in range(FT):
                        st = sink_pool.tile([128, MC], dt.float16, tag="sink")
                        nc.scalar.copy(st[:], spl_t[:, ob % FT, :])
                        nc.sync.dma_start(out[128 * ob : 128 * (ob + 1), :], st[:])

    nc.compile()
    _GRAPH_CACHE[key] = nc
    return nc


# --------------------------------------------------------------------------
# Host-side parameter preparation
# --------------------------------------------------------------------------


def _prep(x, grid, coeffs, knot_alive, proj_w, proj_b, res_w, res_split=False):
    g64 = grid.astype(np.float64)
    order = np.argsort(g64, axis=1, kind="stable")
    sg = np.take_along_axis(grid.astype(np.float32), order, axis=1)
    mcu = coeffs.astype(np.float32) * (
        1.0 / (1.0 + np.exp(-knot_alive.astype(np.float32)))
    )
    mc = np.take_along_axis(mcu, order, axis=1).astype(np.float64)  # (IN, K)

    gmin = sg[:, 0].astype(np.float64)
    gmax = sg[:, -1].astype(np.float64)
    rng = np.maximum(gmax - gmin, 1e-6)
    gscale = (K - 1) / rng  # (IN,)
    gbias = -gmin * gscale

    # kink decomposition: f(w) = mc0 + sum_{j=0..11} D_j relu(w - j)
    s = mc[:, 1:] - mc[:, :-1]  # slopes, (IN, 11)
    D = np.empty((IN, NKINK), dtype=np.float64)
    D[:, 0] = s[:, 0]
    D[:, 1:11] = s[:, 1:] - s[:, :-1]
    D[:, 11] = -s[:, -1]
    Dd = D * SPL_SCALE

    # normalized coordinate, feature-major
    w = (x.astype(np.float64) * gscale[None, :] + gbias[None, :]).T  # (IN, M)
    # spline coordinate: clamped (f(clip(w)) == f(w) exactly); the res path
    # needs the UNCLAMPED coordinate, shipped separately
    w16 = np.ascontiguousarray(np.clip(w, 0.0, 11.0), dtype=F16)
    w16u = np.ascontiguousarray(w, dtype=F16)

    pwT = proj_w.astype(np.float64).T  # (IN, OUT)
    rwT = res_w.astype(np.float64).T  # (IN, OUT)
    vres = rwT / gscale[:, None]

    def _tile_rows(a, dtype):
        return np.ascontiguousarray(
            a.reshape(FT, 128, OUT).transpose(1, 0, 2), dtype=dtype
        )

    def _tile_cols(a, dtype):
        # (IN, C) -> [128, FT, C]
        C = a.shape[1]
        return np.ascontiguousarray(
            a.reshape(FT, 128, C).transpose(1, 0, 2), dtype=dtype
        )

    # host-side bias (added after gather): proj_b + mc0 @ pwT + gmin @ rwT
    hbias = proj_b.astype(np.float64) + mc[:, 0] @ pwT + gmin @ rwT
    if res_split:
        hbias = hbias + W_CENTER * vres.sum(axis=0)

    sig = np.where(D < 0, -1.0, 1.0)  # (IN, K)
    aDd = np.abs(Dd)

    # ACT3 kink params (scale |Dd_j|, bias -j*|Dd_j|)
    ks = list(ACT3)
    if ks:
        asc = np.stack([aDd[:, j] for j in ks], axis=1)  # (IN, N3)
        abi = np.stack([-j * aDd[:, j] for j in ks], axis=1)
    else:
        asc = np.zeros((IN, 0)); abi = np.zeros((IN, 0))

    wtb = _tile_rows(pwT / SPL_SCALE, FP8)
    # per-ACT3-kink signed weights: [128, N3, FT, OUT]
    if ACT3:
        wpe = np.stack(
            [_tile_rows(sig[:, j : j + 1] * pwT / SPL_SCALE, FP8) for j in ACT3],
            axis=1,
        )
        wpe = np.ascontiguousarray(wpe)
    else:
        wpe = np.zeros((128, 0, FT, OUT), dtype=FP8)

    dcoef = _tile_cols(Dd, np.float32)
    acts_a = _tile_cols(asc, np.float32)
    actb_a = _tile_cols(abi, np.float32)
    lin0_a = np.ascontiguousarray(
        Dd[:, 0].reshape(FT, 128).T, dtype=np.float32
    )

    parts = {
        "w16": w16,
        "wtb": wtb,
        "lin0": lin0_a,
        "dcoef": dcoef,
    }
    if not res_split:
        parts["w16u"] = w16u
    if ACT3:
        parts["wpe"] = wpe
        parts["acts"] = acts_a
        parts["actb"] = actb_a
    if res_split:
        wc = (w.astype(np.float32) - W_CENTER) / RS
        w8h = np.asarray(wc, FP8)
        w8l = np.asarray(wc - w8h.astype(np.float32), FP8)
        rwh = np.asarray(RS * vres, FP8)
        rwl = np.asarray(RS * vres - rwh.astype(np.float64), FP8)
        parts["w8h"] = w8h
        parts["w8l"] = w8l
        parts["rwh"] = _tile_rows(rwh.astype(np.float32), FP8)
        parts["rwl"] = _tile_rows(rwl.astype(np.float32), FP8)
    else:
        parts["wt8"] = _tile_rows(vres, F16)
    return parts, hbias.astype(np.float32)


_SHARED = ("wtb", "wpe", "acts", "actb", "lin0", "dcoef", "rwh", "rwl", "wt8")


def _make_in_maps(inputs, res_split=False):
    parts, _ = _prep(**inputs, res_split=res_split)
    in_maps = []
    for c in range(N_CORES):
        sl = slice(c * MC, (c + 1) * MC)
        m = {}
        for k, v in parts.items():
            m[k] = v if k in _SHARED else np.ascontiguousarray(v[:, sl])
        in_maps.append(m)
    return in_maps


def kernel(**inputs):
    from concourse.bass_utils import run_bass_kernel_spmd

    inputs = {k: np.asarray(v) for k, v in inputs.items()}
    res_split = False
    nc = _build_graph(res_split=res_split)
    parts, hbias = _prep(**inputs, res_split=res_split)
    in_maps = []
    for c in range(N_CORES):
        sl = slice(c * MC, (c + 1) * MC)
        m = {}
        for k, v in parts.items():
            m[k] = v if k in _SHARED else np.ascontiguousarray(v[:, sl])
        in_maps.append(m)
    res = run_bass_kernel_spmd(nc, in_maps, core_ids=list(range(N_CORES)))
    return np.concatenate(
        [
            res.results[c]["out"].T.astype(np.float32) + hbias[None, :]
            for c in range(N_CORES)
        ],
        axis=0,
    )


if __name__ == "__main__":
    rng = np.random.default_rng(0)
    fake = {
        "x": rng.standard_normal((M, IN), dtype=np.float32),
        "grid": rng.standard_normal((IN, K), dtype=np.float32),
        "coeffs": rng.standard_normal((IN, K), dtype=np.float32) * 0.1,
        "knot_alive": rng.standard_normal((IN, K), dtype=np.float32) + 3,
        "proj_w": rng.standard_normal((OUT, IN), dtype=np.float32) / 32,
        "proj_b": rng.standard_normal((OUT,), dtype=np.float32) * 0.01,
        "res_w": rng.standard_normal((IN, OUT), dtype=np.float32).T / 32,
    }
    y = kernel(**fake)
    print("kernel output", y.shape, y.dtype)
